# revision 1
# baseline (speedup 1.0000x reference)
# MemN2N forward kernel for Trainium2 (8 NeuronCores, Bass/Tile).
#
# Problem: B=256, V=50000, E=512, S=3 sentence slots, M=200 memories,
# HOPS=3, C=7 classes, D=S*E=1536.
#
# Sharding: data-parallel over batch, 32 batches per core. The embedding
# table is replicated; per core it is compacted to the tokens that core
# actually uses (so gather indices fit in int16 for dma_gather) and
# pre-scaled by the (deterministic) position encoding, one table per
# sentence slot. The 7 fc_w rows are appended to each table so the whole
# extended system below is produced by a single gather per (group, slot).
#
# Algorithm (per batch b):
#   m  = emb[stories_b] * enc         (200, 1536)   -- the expensive gather
#   u0 = emb[queries_b] * enc         (1536,)
#   mt = [m; u0; fc_w]                (208, 1536)
#   One Gram matrix G = mt @ mt.T (208x208) contains every inner product
#   the 3 hops need:
#     dotted_0   = G[200, :200]                  (= m @ u0)
#     dotted_h+1 = dotted_h + G[:200,:200] @ p_h
#     y          = G[200, 201:208] + (p0+p1+p2) @ G[:200, 201:208] + fc_b
#   so the gathered memories are read from HBM exactly once.
#
# On device, a PSUM scores tile S[32, 208] accumulates, per batch row b,
#   (e_200 + p0 + p1 + p2) @ G_b
# via matmuls whose stationary operand is a [K, 32] matrix with only
# column b nonzero (diagonal-embedded p vectors), which lets all 32
# batches share one PSUM tile, keeps softmax batched, and makes the
# final logits fall out of PSUM columns 201..207.

import numpy as np
import ml_dtypes

# ---- problem constants (hardcoded; kernel.py must be self-contained) ----
B, V, E, S, M, HOPS, C = 256, 50000, 512, 3, 200, 3, 7
D = S * E                   # 1536
NCORES = 8
BL = B // NCORES            # 32 batches per core
GB = 4                      # batches per gather group
NG = BL // GB               # 8 groups
NR = M + 1 + C              # 208 rows of the extended Gram system
# gather indices per (group, slot): GB*NR rounded up to a multiple of 128.
# (transposed dma_gather writes 4*num_idxs*2 bytes per partition and is
# limited to 8KB there: num_idxs <= 1024. 1664 crashes the device.)
NIDX = (GB * NR + 127) // 128 * 128     # 896
KCH = D // 128              # 12 contraction chunks
NLO = NR - 128              # 80 rows in the low Gram block
NQUEUES = 4                 # SWDGE queues for gather descriptor generation

BF16 = ml_dtypes.bfloat16

_CACHE = {}


def _position_encoding(sentence_size, embedding_size):
    i = np.arange(1, embedding_size + 1, dtype=np.float32)[:, None]
    j = np.arange(1, sentence_size + 1, dtype=np.float32)[None, :]
    le, ls = embedding_size + 1, sentence_size + 1
    enc = (i - (le - 1) / 2.0) * (j - (ls - 1) / 2.0)
    enc = 1.0 + 4.0 * enc / embedding_size / sentence_size
    return np.transpose(enc).astype(np.float32)


def _build_program(dpad, stage="full"):
    import concourse.bacc as bacc
    import concourse.bass as bass
    import concourse.mybir as mybir
    import concourse.tile as tile
    from concourse.masks import make_identity

    dt = mybir.dt
    nc = bacc.Bacc("TRN2", target_bir_lowering=False, debug=False,
                   num_swdge_queues=NQUEUES)
    dbg_t = None
    if stage != "full":
        dbg_t = nc.dram_tensor("dbg", [128, BL, NR], dt.float32,
                               kind="ExternalOutput")

    emb_t = [
        nc.dram_tensor(f"emb{s}", [dpad, E], dt.bfloat16, kind="ExternalInput")
        for s in range(S)
    ]
    idxm_t = nc.dram_tensor("idxm", [128, NG * S, NIDX // 16], dt.int16,
                            kind="ExternalInput")
    fcb_t = nc.dram_tensor("fcb", [BL, C], dt.float32, kind="ExternalInput")
    e1m_t = nc.dram_tensor("e1m", [NLO, 32 * 32], dt.bfloat16,
                           kind="ExternalInput")
    y_t = nc.dram_tensor("y", [BL, C], dt.float32, kind="ExternalOutput")

    with tile.TileContext(nc) as tc:
        with (
            tc.tile_pool(name="const", bufs=1) as cpool,
            tc.tile_pool(name="gath", bufs=3) as gpool,
            tc.tile_pool(name="gram", bufs=1) as grpool,
            tc.tile_pool(name="work", bufs=2) as wpool,
            tc.tile_pool(name="psum", bufs=2, space="PSUM") as ppool,
            tc.tile_pool(name="psT", bufs=1, space="PSUM") as tpool,
            tc.tile_pool(name="psS", bufs=1, space="PSUM") as spool,
        ):
            # ---- constants / small inputs ----
            idm = cpool.tile([128, NG * S, NIDX // 16], dt.int16)
            nc.sync.dma_start(idm[:], idxm_t[:])
            fcb = cpool.tile([BL, C], dt.float32)
            nc.sync.dma_start(fcb[:], fcb_t[:])

            ident = cpool.tile([32, 32], dt.bfloat16)
            make_identity(nc, ident[:])

            # e200 selector: [NLO, 32*32] with [72, b*33] = 1 -> stationary
            # operand that routes G_b[200, :] into scores row b. Built on
            # host (single-partition writes at partition 72 are not legal
            # compute-engine APs).
            e1m = cpool.tile([NLO, 32 * 32], dt.bfloat16)
            nc.sync.dma_start(e1m[:], e1m_t[:])

            grh = grpool.tile([128, BL, NR], dt.bfloat16)
            grl = grpool.tile([NLO, BL, NR], dt.bfloat16)

            # ---- main pipeline: gather group -> Gram ----
            for g in range(NG):
                mts = []
                for s in range(S):
                    mt = gpool.tile([128, 4, NIDX], dt.bfloat16, tag=f"mt{s}")
                    nc.gpsimd.dma_gather(
                        mt[:],
                        emb_t[s][:, :],
                        idm[:, g * S + s, :],
                        NIDX, NIDX, E,
                        transpose=True,
                        queue_num=(g * S + s) % NQUEUES,
                    )
                    mts.append(mt)

                if stage == "gather":
                    if g == 0:
                        t = mts[0][:]
                        view = bass.AP(
                            t.tensor, t.offset,
                            [t.ap[0], t.ap[1], [NR, GB], [1, NR]],
                        )
                        dbgs = wpool.tile([128, 4 * GB, NR], dt.float32,
                                          tag="dbgs")
                        nc.vector.tensor_copy(
                            dbgs[:].rearrange("p (c b) r -> p c b r", c=4),
                            view)
                        nc.sync.dma_start(dbg_t[:, 0:4 * GB, :], dbgs[:])
                    continue

                for b8 in range(GB):
                    bg = g * GB + b8
                    ph = ppool.tile([128, NR], dt.float32, tag="ph")
                    pl = ppool.tile([NLO, NR], dt.float32, tag="pl")
                    for k in range(KCH):
                        s, c = k // 4, k % 4
                        base = mts[s][:, c, b8 * NR:(b8 + 1) * NR]
                        nc.tensor.matmul(
                            ph[:], lhsT=base[:, 0:128], rhs=base,
                            start=(k == 0), stop=(k == KCH - 1),
                        )
                        nc.tensor.matmul(
                            pl[:], lhsT=base[:, 128:NR], rhs=base,
                            start=(k == 0), stop=(k == KCH - 1),
                        )
                    nc.scalar.copy(grh[:, bg, :], ph[:])
                    nc.scalar.copy(grl[:, bg, :], pl[:])

            if stage == "gram":
                dbgs = wpool.tile([128, BL, NR], dt.float32, tag="dbgs")
                nc.vector.tensor_copy(dbgs[:], grh[:])
                nc.sync.dma_start(dbg_t[:], dbgs[:])

            # ---- hops ----
            do_hops = stage in ("full", "hops1")
            nhops = HOPS if stage == "full" else 1
            if do_hops:
                Sc = spool.tile([BL, NR], dt.float32)
                for b in range(BL):
                    nc.tensor.matmul(
                        Sc[:], lhsT=e1m[:, b * 32:(b + 1) * 32],
                        rhs=grl[:, b, :],
                        start=(b == 0), stop=False, skip_group_check=True,
                    )
            for h in range(nhops if do_hops else 0):
                eexp = wpool.tile([BL, M], dt.float32, tag="eexp")
                sume = wpool.tile([BL, 1], dt.float32, tag="sume")
                nc.scalar.activation(
                    eexp[:], Sc[:, 0:M],
                    mybir.ActivationFunctionType.Exp,
                    accum_out=sume[:],
                )
                rs = wpool.tile([BL, 1], dt.float32, tag="rs")
                nc.vector.reciprocal(rs[:], sume[:])
                pbf = wpool.tile([BL, M], dt.bfloat16, tag="pbf")
                nc.vector.tensor_scalar_mul(pbf[:], eexp[:], rs[:])

                pth = tpool.tile([128, 32], dt.bfloat16, tag="pth")
                ptl = tpool.tile([M - 128, 32], dt.bfloat16, tag="ptl")
                nc.tensor.transpose(pth[:], pbf[:, 0:128], ident[:])
                nc.tensor.transpose(ptl[:], pbf[:, 128:M], ident[:])

                pm0 = wpool.tile([128, 32 * 32], dt.bfloat16, tag="pm0")
                pm1 = wpool.tile([NLO, 32 * 32], dt.bfloat16, tag="pm1")
                nc.vector.memset(pm0[:], 0.0)
                nc.vector.memset(pm1[:], 0.0)
                nc.vector.tensor_copy(pm0[:, ::33], pth[:])
                nc.vector.tensor_copy(pm1[0:M - 128, ::33], ptl[:])

                last = h == nhops - 1
                for b in range(BL):
                    nc.tensor.matmul(
                        Sc[:], lhsT=pm0[:, b * 32:(b + 1) * 32],
                        rhs=grh[:, b, :],
                        start=False, stop=False, skip_group_check=True,
                    )
                    nc.tensor.matmul(
                        Sc[:], lhsT=pm1[:, b * 32:(b + 1) * 32],
                        rhs=grl[:, b, :],
                        start=False, stop=(last and b == BL - 1),
                        skip_group_check=True,
                    )

            yt = wpool.tile([BL, C], dt.float32, tag="yt")
            if do_hops:
                nc.vector.tensor_add(yt[:], Sc[:, M + 1:NR], fcb[:])
                if stage == "hops1":
                    dbgs = wpool.tile([128, BL, NR], dt.float32, tag="dbgs")
                    nc.vector.memset(dbgs[:], 0.0)
                    nc.vector.tensor_copy(dbgs[0:BL, 0, :], Sc[:])
                    nc.sync.dma_start(dbg_t[:], dbgs[:])
            else:
                nc.vector.memset(yt[:], 0.0)
            nc.sync.dma_start(y_t[:], yt[:])

    nc.compile()
    return nc


def _wrap16(lst):
    """Index list -> dma_gather layout: [16, n/16] with logical i at
    [i % 16, i // 16], replicated to 128 partitions."""
    a = np.asarray(lst, dtype=np.int16)
    assert a.size % 16 == 0
    a2 = a.reshape(-1, 16).T.copy()
    return np.tile(a2, (8, 1))


def _prepare_core_inputs(stories, queries, emb, fc_w, fc_b, enc):
    """Host-side shard prep: per-core token compaction + index layouts.

    Each per-slot table holds the enc-scaled embedding rows for this
    core's tokens, followed by the 7 raw fc_w feature slices for that
    slot, so one gather per (group, slot) produces complete rows of the
    extended system [m; u0; fc_w]."""
    per_core = []
    toks_list = []
    for cid in range(NCORES):
        st = stories[cid * BL:(cid + 1) * BL]
        qu = queries[cid * BL:(cid + 1) * BL]
        toks = np.unique(np.concatenate([st.ravel(), qu.ravel()]))
        toks_list.append(toks)
    dpad = max(len(t) for t in toks_list) + C
    dpad = (dpad + 127) // 128 * 128

    fcb_rep = np.tile(fc_b[None, :], (BL, 1)).astype(np.float32)
    e1m = np.zeros((NLO, 32 * 32), dtype=BF16)
    e1m[M - 128, ::33] = 1.0

    for cid in range(NCORES):
        st = stories[cid * BL:(cid + 1) * BL]
        qu = queries[cid * BL:(cid + 1) * BL]
        toks = toks_list[cid]
        ntok = len(toks)
        inv = np.zeros(V, dtype=np.int64)
        inv[toks] = np.arange(ntok)

        sub = emb[toks].astype(np.float32)
        embs = []
        for s in range(S):
            tbl = np.zeros((dpad, E), dtype=BF16)
            tbl[:ntok] = (sub * enc[s * E:(s + 1) * E][None, :]).astype(BF16)
            tbl[ntok:ntok + C] = fc_w[:, s * E:(s + 1) * E].astype(BF16)
            embs.append(tbl)

        sidx = inv[st]          # (BL, M, S)
        qidx = inv[qu]          # (BL, S)
        fcrows = ntok + np.arange(C)

        idxm = np.zeros((128, NG * S, NIDX // 16), dtype=np.int16)
        for g in range(NG):
            for s in range(S):
                lst = np.zeros(NIDX, dtype=np.int64)
                blk = lst[:GB * NR].reshape(GB, NR)
                blk[:, :M] = sidx[g * GB:(g + 1) * GB, :, s]
                blk[:, M] = qidx[g * GB:(g + 1) * GB, s]
                blk[:, M + 1:] = fcrows[None, :]
                idxm[:, g * S + s, :] = _wrap16(lst)

        in_map = {
            "emb0": embs[0], "emb1": embs[1], "emb2": embs[2],
            "idxm": idxm, "fcb": fcb_rep, "e1m": e1m,
        }
        per_core.append(in_map)
    return dpad, per_core


def kernel(stories, queries, emb, fc_w, fc_b, _trace=False):
    from concourse import bass_utils

    stories = np.asarray(stories)
    queries = np.asarray(queries)
    emb = np.asarray(emb, dtype=np.float32)
    fc_w = np.asarray(fc_w, dtype=np.float32)
    fc_b = np.asarray(fc_b, dtype=np.float32)

    enc = _position_encoding(1, D).reshape(D)
    dpad, in_maps = _prepare_core_inputs(stories, queries, emb, fc_w, fc_b, enc)

    if _CACHE.get("dpad") != dpad:
        _CACHE["nc"] = _build_program(dpad)
        _CACHE["dpad"] = dpad
    nc = _CACHE["nc"]

    res = bass_utils.run_bass_kernel_spmd(
        nc, in_maps, core_ids=list(range(NCORES)), trace=_trace,
    )
    out = np.concatenate([r["y"] for r in res.results], axis=0)
    if _trace:
        _CACHE["last_exec_time_ns"] = res.exec_time_ns
        _CACHE["last_mean_exec_time_ns"] = res.mean_exec_time_ns
    return out.astype(np.float32)

